# revision 5
# baseline (speedup 1.0000x reference)
"""Sharded MHA-with-RoPE Trainium2 kernel (nn_CustomTorchMHASelf).

Contract: kernel(**inputs) takes the FULL unsharded inputs of the
reference (x [2,2048,2048], Wqkv_w [6144,2048], Wqkv_b [6144],
out_w [2048,2048], out_b [2048]) and returns the full [2,2048,2048]
fp32 output, running the compute on 8 NeuronCores.

Sharding: core = b*4 + g handles batch b and head-group g (4 of the 16
heads). Each core computes q/k/v projections for its heads, RoPE,
softmax attention, and its slice of the out-projection; the host sums
the 4 partial outputs per batch and adds out_b.

Device data plane is bf16 (fp32 PSUM accumulation). v3 structure:
 - QKV bias is folded into the projection matmul as a 17th contraction
   block whose x-side is a constant ones row (kills the DVE bias op).
 - The post-matmul psum->SBUF move for the RoPE half-swap DMA runs on
   the otherwise-idle Scalar engine (ACT) in phase A, as do the v
   copies; DVE only does the 2 RoPE multiplies + 1 add per feature
   block (adds pair-merged across two psum banks).
 - exp is pair-merged: one ACT instruction covers two 512-wide score
   banks, halving ACT's fixed per-instruction overhead. exp computes
   exp(s/sqrt(D) - 2); the shift cancels between numerator and
   denominator.
 - The out-projection for a sequence chunk is emitted as soon as the
   last head's attention for that chunk completes, overlapping phase C
   with phase B's tail.
 - PSUM: phase A 2x[2-bank qk] + 2x[2-bank v]; phase B 2x[2-bank
   scores] + psc + psd + 2x[1-bank out-proj] = 8 banks.
"""

import math
import os
import sys
import types

import numpy as np
import ml_dtypes

import concourse.bass as bass
import concourse.mybir as mybir
import concourse.tile as tile
from concourse.bass import ds

F32 = mybir.dt.float32
BF16 = mybir.dt.bfloat16
Alu = mybir.AluOpType
Act = mybir.ActivationFunctionType
BF = ml_dtypes.bfloat16

S, E, HTOT, HL, D, P = 2048, 2048, 16, 4, 128, 128

# Filled with the profile exec time (ns) when MHA_TRACE=1; read by test.py.
LAST_EXEC_NS = None


def _install_axon_ntff_shim():
    """Provide antenv.axon_hooks so trace=True can reach the axon NTFF hook."""
    if "antenv.axon_hooks" in sys.modules:
        return
    mod = types.ModuleType("antenv.axon_hooks")
    holder = [None]
    mod.set_axon_ntff_profile_hook = lambda h: holder.__setitem__(0, h)
    mod.get_axon_ntff_profile_hook = lambda: holder[0]
    sys.modules["antenv.axon_hooks"] = mod
    try:
        import antenv
        antenv.axon_hooks = mod
    except ImportError:
        pass
    # boot() ran at interpreter start (sitecustomize), before this module
    # existed, so its NTFF-hook registration was silently skipped. Redo it.
    try:
        from trn_agent_boot.trn_boot import _ntff_profile_via_ctypes
        hook = _ntff_profile_via_ctypes("/opt/axon/libaxon_pjrt.so")
        if hook is not None:
            mod.set_axon_ntff_profile_hook(hook)
    except Exception:
        pass


def _split_multi_waits(nc):
    """Hoist extra sem-waits onto standalone NoOps (one wait per inst).

    This walrus build rejects any instruction carrying more than one
    sync-wait ("Too many sync wait commands"); Tile attaches one wait per
    outstanding semaphore to the consuming instruction. Splitting them
    across same-engine NoOps placed immediately before is equivalent:
    the engine executes serially, so all waits still precede the inst.
    """
    ctr = 0
    for fn in nc.m.functions:
        for blk in fn.blocks:
            out = []
            for inst in blk.instructions:
                si = getattr(inst, "sync_info", None)
                if si is not None and si.on_wait is not None \
                        and len(si.on_wait) > 1:
                    waits = list(si.on_wait)
                    si.on_wait = [waits[-1]]
                    for w in waits[:-1]:
                        ctr += 1
                        nop = mybir.InstNoOp(
                            name=f"I-wsplit-{ctr}", ins=[], outs=[])
                        nop.engine = inst.engine
                        nop.sync_info = mybir.SyncInfo(
                            on_wait=[w], on_update=[])
                        out.append(nop)
                out.append(inst)
            blk.instructions[:] = out


def _build_mha(nc: bass.Bass):
    """Emit the per-core MHA program (one shard) into `nc`."""
    EO = E // P            # 16 contraction subtiles for the projections
    NQK = 2 * HL           # 8 q/k feature blocks (q_h,k_h interleaved)
    ST = 512               # free-dim tile (one PSUM bank of fp32)
    NS = S // ST           # 4 seq chunks
    SB = S // P            # 16 token blocks
    JT = S // P            # 16 key blocks per head
    JP = JT // 2           # 8 key-block pairs
    ET = E // ST           # 4 out-proj column tiles
    H = D // 2

    xT = nc.dram_tensor("xT", [E, S], BF16, kind="ExternalInput")
    wqkT = nc.dram_tensor("wqkT", [(EO + 1) * P, NQK * D], BF16,
                          kind="ExternalInput")
    wvT = nc.dram_tensor("wvT", [(EO + 1) * P, HL * D], BF16,
                         kind="ExternalInput")
    xones = nc.dram_tensor("xones", [P, ST], BF16, kind="ExternalInput")
    ones = nc.dram_tensor("ones", [P, P], BF16, kind="ExternalInput")
    cosT = nc.dram_tensor("cosT", [D, S], F32, kind="ExternalInput")
    sinT = nc.dram_tensor("sinT", [D, S], F32, kind="ExternalInput")
    owT = nc.dram_tensor("owT", [HL * D, E], BF16, kind="ExternalInput")
    out = nc.dram_tensor("out", [S, E], F32, kind="ExternalOutput")

    isc = 1.0 / math.sqrt(D)

    from contextlib import ExitStack

    with tile.TileContext(nc) as tc, ExitStack() as stk:
        persist = stk.enter_context(tc.tile_pool(name="persist", bufs=1))
        qkT_sb = persist.tile([P, NQK, S], BF16)    # post-RoPE q/k [d, jb, s]
        v_sb = persist.tile([P, SB, HL * D], BF16)  # v natural [s%128, s//128, hd]
        ones_sb = persist.tile([P, P], BF16)
        xones_sb = persist.tile([P, ST], BF16)
        cos_sb = persist.tile([P, S], F32)
        sin_sb = persist.tile([P, S], F32)
        negtwo = persist.tile([P, 1], F32)
        nc.gpsimd.memset(negtwo[:], -2.0)
        nc.sync.dma_start(ones_sb[:], ones[:])
        nc.sync.dma_start(xones_sb[:], xones[:])
        nc.sync.dma_start(cos_sb[:], cosT[:])
        nc.sync.dma_start(sin_sb[:], sinT[:])

        # ---- Phase A: QKV projection + folded bias + RoPE ----
        with tc.tile_pool(name="phaseA", bufs=1) as pa, \
             tc.tile_pool(name="xstream", bufs=2) as xs, \
             tc.tile_pool(name="ropebf", bufs=2) as rb, \
             tc.tile_pool(name="ropetmp", bufs=2) as rt, \
             tc.tile_pool(name="psQK", bufs=2, space="PSUM") as psQK, \
             tc.tile_pool(name="psV", bufs=2, space="PSUM") as psV:
            wqk_sb = pa.tile([P, EO + 1, NQK * D], BF16)
            wv_sb = pa.tile([P, EO + 1, HL * D], BF16)
            # interleave the first x-slice with the weights so the first
            # matmuls (which consume eo=0 tiles) aren't queued behind all
            # the weight DMA
            xt0 = xs.tile([P, EO, ST], BF16, tag="xt", name="xt0")
            for eo in range(EO + 1):
                nc.sync.dma_start(wqk_sb[:, eo, :], wqkT[ds(eo * P, P), :])
                if eo < EO:
                    nc.sync.dma_start(
                        xt0[:, eo, :], xT[ds(eo * P, P), ds(0, ST)])
                nc.sync.dma_start(wv_sb[:, eo, :], wvT[ds(eo * P, P), :])

            for i in range(NS):
                if i == 0:
                    xt = xt0
                else:
                    xt = xs.tile([P, EO, ST], BF16, tag="xt")
                    for eo in range(EO):
                        nc.sync.dma_start(
                            xt[:, eo, :], xT[ds(eo * P, P), ds(i * ST, ST)])
                sl = ds(i * ST, ST)
                for jbp in range(HL):       # pair (q_h, k_h) for head jbp
                    ps = psQK.tile([P, 2, ST], F32, tag="qkacc")
                    for s in range(2):
                        jb = 2 * jbp + s
                        for eo in range(EO):
                            nc.tensor.matmul(
                                ps[:, s, :],
                                wqk_sb[:, eo, ds(jb * D, D)], xt[:, eo, :],
                                start=(eo == 0), stop=False)
                        nc.tensor.matmul(
                            ps[:, s, :],
                            wqk_sb[:, EO, ds(jb * D, D)], xones_sb[:],
                            start=False, stop=True)
                    # RoPE: psum holds qb = q + bias. ACT copies it to SBUF
                    # (bf16) for the half-swap DMA; out = qb*cos + rot*sinSW
                    # with the rotation sign folded into the host sin table.
                    qb = rb.tile([P, 2, ST], BF16, tag="qb")
                    nc.scalar.activation(qb[:], ps[:], Act.Copy)
                    rot = rb.tile([P, 2, ST], BF16, tag="rot")
                    nc.sync.dma_start(rot[:H], qb[H:])
                    nc.sync.dma_start(rot[H:], qb[:H])
                    t1 = rt.tile([P, 2, ST], F32, tag="t1")
                    t2 = rt.tile([P, 2, ST], F32, tag="t2")
                    for s in range(2):
                        nc.vector.tensor_tensor(
                            t1[:, s, :], ps[:, s, :], cos_sb[:, sl], Alu.mult)
                        nc.vector.tensor_tensor(
                            t2[:, s, :], rot[:, s, :], sin_sb[:, sl],
                            Alu.mult)
                    nc.vector.tensor_tensor(
                        qkT_sb[:, ds(2 * jbp, 2), sl], t1[:], t2[:], Alu.add)
                for tp in range(2):         # pairs of token blocks
                    psv = psV.tile([P, 2, ST], F32, tag="vacc")
                    for s in range(2):
                        sbl = 2 * tp + s
                        for eo in range(EO):
                            nc.tensor.matmul(
                                psv[:, s, :],
                                xt[:, eo, ds(sbl * P, P)], wv_sb[:, eo, :],
                                start=(eo == 0), stop=False)
                        nc.tensor.matmul(
                            psv[:, s, :],
                            xones_sb[:, ds(0, P)], wv_sb[:, EO, :],
                            start=False, stop=True)
                    nc.scalar.activation(
                        v_sb[:, ds(4 * i + 2 * tp, 2), :], psv[:], Act.Copy)

        # ---- Phase B: attention per head (+ phase C overlapped on h=3) ----
        with tc.tile_pool(name="phaseB", bufs=3) as pb, \
             tc.tile_pool(name="bweights", bufs=1) as bw, \
             tc.tile_pool(name="recipp", bufs=2) as rp, \
             tc.tile_pool(name="ocopy", bufs=3) as oc, \
             tc.tile_pool(name="psS", bufs=2, space="PSUM") as psS, \
             tc.tile_pool(name="pscp", bufs=1, space="PSUM") as pscp, \
             tc.tile_pool(name="psdp", bufs=1, space="PSUM") as psdp, \
             tc.tile_pool(name="psO", bufs=2, space="PSUM") as psO:
            ctxT_sb = bw.tile([P, HL, S], BF16)     # [d, h, i]
            ow_sb = bw.tile([P, HL, E], BF16)
            for ho in range(HL):
                nc.sync.dma_start(ow_sb[:, ho, :], owT[ds(ho * P, P), :])
            for h in range(HL):
                for i in range(NS):
                    psc = pscp.tile([P, ST], F32, tag="psc")
                    psd = psdp.tile([P, ST], F32, tag="psd")
                    for jp in range(JP):
                        megaS = psS.tile([P, 2, ST], F32, tag="megaS")
                        for s in range(2):
                            jb = 2 * jp + s
                            nc.tensor.matmul(
                                megaS[:, s, :],
                                qkT_sb[:, 2 * h + 1, ds(jb * P, P)],
                                qkT_sb[:, 2 * h, ds(i * ST, ST)],
                                start=True, stop=True)
                        att = pb.tile([P, 2, ST], BF16, tag="att")
                        nc.scalar.activation(
                            att[:], megaS[:], Act.Exp, scale=isc,
                            bias=negtwo[:])
                        for s in range(2):
                            jb = 2 * jp + s
                            nc.tensor.matmul(
                                psc[:], v_sb[:, jb, ds(h * D, D)],
                                att[:, s, :],
                                start=(jp == 0 and s == 0),
                                stop=(jp == JP - 1 and s == 1))
                            nc.tensor.matmul(
                                psd[:], ones_sb[:], att[:, s, :],
                                start=(jp == 0 and s == 0),
                                stop=(jp == JP - 1 and s == 1))
                    rec = rp.tile([P, ST], F32, tag="rec")
                    nc.vector.reciprocal(rec[:], psd[:])
                    nc.vector.tensor_tensor(
                        ctxT_sb[:, h, ds(i * ST, ST)], psc[:], rec[:],
                        Alu.mult)
                    # ---- Phase C for this seq chunk (all heads now done) --
                    if h == HL - 1:
                        for sb in range(4 * i, 4 * i + 4):
                            for et in range(ET):
                                pso = psO.tile([P, ST], F32, tag="pso")
                                for ho in range(HL):
                                    nc.tensor.matmul(
                                        pso[:],
                                        ctxT_sb[:, ho, ds(sb * P, P)],
                                        ow_sb[:, ho, ds(et * ST, ST)],
                                        start=(ho == 0), stop=(ho == HL - 1))
                                ot = oc.tile([P, ST], F32, tag="ot")
                                nc.vector.tensor_copy(ot[:], pso[:])
                                nc.sync.dma_start(
                                    out[ds(sb * P, P), ds(et * ST, ST)],
                                    ot[:])

    return nc


def _rope_tables():
    inv_freq = 1.0 / (10000.0 ** (np.arange(0, D, 2, dtype=np.float32) / D))
    t = np.arange(S, dtype=np.float32)
    freqs = np.einsum("s,f->sf", t, inv_freq)
    emb = np.concatenate([freqs, freqs], axis=-1)
    cosT = np.cos(emb).astype(np.float32).T.copy()
    sinT = np.sin(emb).astype(np.float32).T.copy()
    # fold the rotate-half sign in: out = qb*cos + halfswap(qb)*sinSW
    sinSW = np.concatenate([-sinT[:D // 2], sinT[D // 2:]], axis=0)
    return cosT, np.ascontiguousarray(sinSW)


def _core_inputs(Wqkv_w, Wqkv_b, out_w, g, cosT, sinT, xT_bf, xones, ones):
    qk_cols, qkb = [], []
    for hl in range(HL):
        h = g * HL + hl
        qk_cols.append(Wqkv_w[h * D:(h + 1) * D, :].T)
        qk_cols.append(Wqkv_w[E + h * D:E + (h + 1) * D, :].T)
        qkb.append(Wqkv_b[h * D:(h + 1) * D])
        qkb.append(Wqkv_b[E + h * D:E + (h + 1) * D])
    wqkT = np.zeros((E + P, 2 * HL * D), dtype=np.float32)
    wqkT[:E] = np.concatenate(qk_cols, axis=1)
    wqkT[E] = np.concatenate(qkb)                 # bias row
    v0 = 2 * E + g * HL * D
    wvT = np.zeros((E + P, HL * D), dtype=np.float32)
    wvT[:E] = Wqkv_w[v0:v0 + HL * D, :].T
    wvT[E] = Wqkv_b[v0:v0 + HL * D]
    owT = np.ascontiguousarray(
        out_w[:, g * HL * D:(g + 1) * HL * D].T).astype(BF)
    return {"xT": xT_bf, "wqkT": wqkT.astype(BF), "wvT": wvT.astype(BF),
            "xones": xones, "ones": ones,
            "cosT": cosT, "sinT": sinT, "owT": owT}


def kernel(x, Wqkv_w, Wqkv_b, out_w, out_b):
    global LAST_EXEC_NS
    _install_axon_ntff_shim()
    from concourse.bass_utils import run_bass_kernel_spmd

    x = np.asarray(x, dtype=np.float32)
    Wqkv_w = np.asarray(Wqkv_w, dtype=np.float32)
    Wqkv_b = np.asarray(Wqkv_b, dtype=np.float32)
    out_w = np.asarray(out_w, dtype=np.float32)
    out_b = np.asarray(out_b, dtype=np.float32)

    cosT, sinT = _rope_tables()
    xT_bf = [np.ascontiguousarray(x[b].T).astype(BF) for b in range(2)]
    xones = np.zeros((P, 512), dtype=BF)
    xones[0, :] = BF(1.0)
    ones = np.ones((P, P), dtype=BF)
    in_maps = []
    for core in range(8):
        b, g = core // 4, core % 4
        in_maps.append(_core_inputs(
            Wqkv_w, Wqkv_b, out_w, g, cosT, sinT, xT_bf[b], xones, ones))

    nc = bass.Bass()
    _build_mha(nc)
    _split_multi_waits(nc)

    trace = bool(os.environ.get("MHA_TRACE"))
    if trace:
        # dev-only profiling path; skip the S3 artifact upload
        import concourse.bass_utils as _bu
        _bu.upload_artifacts = lambda tmpdir: tmpdir
    res = run_bass_kernel_spmd(
        nc, in_maps, core_ids=list(range(8)), trace=trace)
    if trace:
        LAST_EXEC_NS = res.exec_time_ns

    out = np.empty((2, S, E), dtype=np.float32)
    for b in range(2):
        acc = res.results[b * 4 + 0]["out"].astype(np.float32).copy()
        for g in range(1, 4):
            acc += res.results[b * 4 + g]["out"]
        out[b] = acc + out_b[None, :]
    return out


# revision 13
# speedup vs baseline: 1.0655x; 1.0655x over previous
"""Sharded MHA-with-RoPE Trainium2 kernel (nn_CustomTorchMHASelf).

Contract: kernel(**inputs) takes the FULL unsharded inputs of the
reference (x [2,2048,2048], Wqkv_w [6144,2048], Wqkv_b [6144],
out_w [2048,2048], out_b [2048]) and returns the full [2,2048,2048]
fp32 output, running the compute on 8 NeuronCores.

Sharding: core = b*4 + g handles batch b and head-group g (4 of the 16
heads). Each core computes q/k/v projections for its heads, RoPE,
softmax attention, and its slice of the out-projection; the host sums
the 4 partial outputs per batch and adds out_b.

Device data plane is bf16 (fp32 PSUM accumulation). v3 structure:
 - QKV bias is folded into the projection matmul as a 17th contraction
   block whose x-side is a constant ones row (kills the DVE bias op).
 - The post-matmul psum->SBUF move for the RoPE half-swap DMA runs on
   the otherwise-idle Scalar engine (ACT) in phase A, as do the v
   copies; DVE only does the 2 RoPE multiplies + 1 add per feature
   block (adds pair-merged across two psum banks).
 - exp is pair-merged: one ACT instruction covers two 512-wide score
   banks, halving ACT's fixed per-instruction overhead. exp computes
   exp(s/sqrt(D) - 2); the shift cancels between numerator and
   denominator.
 - The out-projection for a sequence chunk is emitted as soon as the
   last head's attention for that chunk completes, overlapping phase C
   with phase B's tail.
 - PSUM: phase A 2x[2-bank qk] + 2x[2-bank v]; phase B 2x[2-bank
   scores] + psc + psd + 2x[1-bank out-proj] = 8 banks.
"""

import math
import os
import sys
import types

import numpy as np
import ml_dtypes

import concourse.bass as bass
import concourse.mybir as mybir
import concourse.tile as tile
from concourse.bass import ds

F32 = mybir.dt.float32
BF16 = mybir.dt.bfloat16
Alu = mybir.AluOpType
Act = mybir.ActivationFunctionType
BF = ml_dtypes.bfloat16

S, E, HTOT, HL, D, P = 2048, 2048, 16, 4, 128, 128

# Filled with the profile exec time (ns) when MHA_TRACE=1; read by test.py.
LAST_EXEC_NS = None


def _install_axon_ntff_shim():
    """Provide antenv.axon_hooks so trace=True can reach the axon NTFF hook."""
    if "antenv.axon_hooks" in sys.modules:
        return
    mod = types.ModuleType("antenv.axon_hooks")
    holder = [None]
    mod.set_axon_ntff_profile_hook = lambda h: holder.__setitem__(0, h)
    mod.get_axon_ntff_profile_hook = lambda: holder[0]
    sys.modules["antenv.axon_hooks"] = mod
    try:
        import antenv
        antenv.axon_hooks = mod
    except ImportError:
        pass
    # boot() ran at interpreter start (sitecustomize), before this module
    # existed, so its NTFF-hook registration was silently skipped. Redo it.
    try:
        from trn_agent_boot.trn_boot import _ntff_profile_via_ctypes
        hook = _ntff_profile_via_ctypes("/opt/axon/libaxon_pjrt.so")
        if hook is not None:
            mod.set_axon_ntff_profile_hook(hook)
    except Exception:
        pass


def _split_multi_waits(nc):
    """Hoist extra sem-waits onto standalone NoOps (one wait per inst).

    This walrus build rejects any instruction carrying more than one
    sync-wait ("Too many sync wait commands"); Tile attaches one wait per
    outstanding semaphore to the consuming instruction. Splitting them
    across same-engine NoOps placed immediately before is equivalent:
    the engine executes serially, so all waits still precede the inst.
    """
    ctr = 0
    for fn in nc.m.functions:
        for blk in fn.blocks:
            out = []
            for inst in blk.instructions:
                si = getattr(inst, "sync_info", None)
                if si is not None and si.on_wait is not None \
                        and len(si.on_wait) > 1:
                    waits = list(si.on_wait)
                    si.on_wait = [waits[-1]]
                    for w in waits[:-1]:
                        ctr += 1
                        nop = mybir.InstNoOp(
                            name=f"I-wsplit-{ctr}", ins=[], outs=[])
                        nop.engine = inst.engine
                        nop.sync_info = mybir.SyncInfo(
                            on_wait=[w], on_update=[])
                        out.append(nop)
                out.append(inst)
            blk.instructions[:] = out


def _build_mha(nc: bass.Bass):
    """Emit the per-core MHA program (one shard) into `nc`."""
    EO = E // P            # 16 contraction subtiles for the projections
    NQK = 2 * HL           # 8 q/k feature blocks (q_h,k_h interleaved)
    ST = 512               # free-dim tile (one PSUM bank of fp32)
    NS = S // ST           # 4 seq chunks
    SB = S // P            # 16 token blocks
    JT = S // P            # 16 key blocks per head
    JP = JT // 2           # 8 key-block pairs
    ET = E // ST           # 4 out-proj column tiles
    H = D // 2

    xT = nc.dram_tensor("xT", [E, S], BF16, kind="ExternalInput")
    wqkT = nc.dram_tensor("wqkT", [EO * P, NQK * D], BF16,
                          kind="ExternalInput")
    qkb = nc.dram_tensor("qkb", [NQK, D], F32, kind="ExternalInput")
    wvT = nc.dram_tensor("wvT", [(EO + 1) * P, HL * D], BF16,
                         kind="ExternalInput")
    xones = nc.dram_tensor("xones", [P, ST], BF16, kind="ExternalInput")
    ones = nc.dram_tensor("ones", [P, P], BF16, kind="ExternalInput")
    cosT = nc.dram_tensor("cosT", [D, S], F32, kind="ExternalInput")
    sinT = nc.dram_tensor("sinT", [D, S], F32, kind="ExternalInput")
    owT = nc.dram_tensor("owT", [HL * D, E], BF16, kind="ExternalInput")
    out = nc.dram_tensor("out", [S, E], F32, kind="ExternalOutput")

    isc = 1.0 / math.sqrt(D)

    from contextlib import ExitStack

    with tile.TileContext(nc) as tc, ExitStack() as stk:
        persist = stk.enter_context(tc.tile_pool(name="persist", bufs=1))
        qkT_sb = persist.tile([P, NQK, S], BF16)    # post-RoPE q/k [d, jb, s]
        v_sb = persist.tile([P, SB, HL * D], BF16)  # v natural [s%128, s//128, hd]
        ones_sb = persist.tile([P, P], BF16)
        xones_sb = persist.tile([P, ST], BF16)
        cos_sb = persist.tile([P, S], F32)
        sin_sb = persist.tile([P, S], F32)
        negtwo = persist.tile([P, 1], F32)
        nc.gpsimd.memset(negtwo[:], -2.0)
        nc.sync.dma_start(ones_sb[:], ones[:])
        nc.sync.dma_start(xones_sb[:], xones[:])
        nc.sync.dma_start(cos_sb[:], cosT[:])
        nc.sync.dma_start(sin_sb[:], sinT[:])

        # ---- Phase A: QKV projection + folded bias + RoPE ----
        with tc.tile_pool(name="phaseA", bufs=1) as pa, \
             tc.tile_pool(name="xstream", bufs=2) as xs, \
             tc.tile_pool(name="ropebf", bufs=2) as rb, \
             tc.tile_pool(name="ropetmp", bufs=2) as rt, \
             tc.tile_pool(name="psQK", bufs=2, space="PSUM") as psQK, \
             tc.tile_pool(name="psV", bufs=2, space="PSUM") as psV:
            wqk_sb = pa.tile([P, EO, NQK * D], BF16)
            wv_sb = pa.tile([P, EO + 1, HL * D], BF16)
            qkb_sb = pa.tile([P, NQK], F32)
            nc.sync.dma_start(qkb_sb[:], qkb[:].rearrange("c d -> d c"))
            # interleave the first x-slice with the weights so the first
            # matmuls (which consume eo=0 tiles) aren't queued behind all
            # the weight DMA
            xt0 = xs.tile([P, EO, ST], BF16, tag="xt", name="xt0")
            for eo in range(EO + 1):
                if eo < EO:
                    nc.sync.dma_start(
                        wqk_sb[:, eo, :], wqkT[ds(eo * P, P), :])
                    nc.sync.dma_start(
                        xt0[:, eo, :], xT[ds(eo * P, P), ds(0, ST)])
                nc.sync.dma_start(wv_sb[:, eo, :], wvT[ds(eo * P, P), :])

            for i in range(NS):
                if i == 0:
                    xt = xt0
                else:
                    xt = xs.tile([P, EO, ST], BF16, tag="xt")
                    for eo in range(EO):
                        nc.sync.dma_start(
                            xt[:, eo, :], xT[ds(eo * P, P), ds(i * ST, ST)])
                sl = ds(i * ST, ST)
                for jbp in range(HL):       # pair (q_h, k_h) for head jbp
                    ps = psQK.tile([P, 2, ST], F32, tag="qkacc")
                    for s in range(2):
                        jb = 2 * jbp + s
                        for eo in range(EO):
                            nc.tensor.matmul(
                                ps[:, s, :],
                                wqk_sb[:, eo, ds(jb * D, D)], xt[:, eo, :],
                                start=(eo == 0), stop=(eo == EO - 1))
                    # RoPE: ACT adds the bias while copying psum to SBUF
                    # (bf16) as qb; the half-swap DMA makes rot; out =
                    # qb*cos + rot*sinSW with the rotation sign folded into
                    # the host sin table.
                    qb = rb.tile([P, 2, ST], BF16, tag="qb")
                    for s in range(2):
                        nc.scalar.activation(
                            qb[:, s, :], ps[:, s, :], Act.Identity,
                            bias=qkb_sb[:, 2 * jbp + s, None])
                    rot = rb.tile([P, 2, ST], BF16, tag="rot")
                    nc.sync.dma_start(rot[:H], qb[H:])
                    nc.sync.dma_start(rot[H:], qb[:H])
                    t1 = rt.tile([P, 2, ST], F32, tag="t1")
                    t2 = rt.tile([P, 2, ST], F32, tag="t2")
                    for s in range(2):
                        nc.vector.tensor_tensor(
                            t1[:, s, :], qb[:, s, :], cos_sb[:, sl],
                            Alu.mult)
                        nc.vector.tensor_tensor(
                            t2[:, s, :], rot[:, s, :], sin_sb[:, sl],
                            Alu.mult)
                    nc.vector.tensor_tensor(
                        qkT_sb[:, ds(2 * jbp, 2), sl], t1[:], t2[:], Alu.add)
                for tp in range(2):         # pairs of token blocks
                    psv = psV.tile([P, 2, ST], F32, tag="vacc")
                    for s in range(2):
                        sbl = 2 * tp + s
                        for eo in range(EO):
                            nc.tensor.matmul(
                                psv[:, s, :],
                                xt[:, eo, ds(sbl * P, P)], wv_sb[:, eo, :],
                                start=(eo == 0), stop=False)
                        nc.tensor.matmul(
                            psv[:, s, :],
                            xones_sb[:, ds(0, P)], wv_sb[:, EO, :],
                            start=False, stop=True)
                    nc.scalar.activation(
                        v_sb[:, ds(4 * i + 2 * tp, 2), :], psv[:], Act.Copy)

        # ---- Phase B: attention per head (+ phase C overlapped on h=3) ----
        with tc.tile_pool(name="phaseB", bufs=3) as pb, \
             tc.tile_pool(name="bweights", bufs=1) as bw, \
             tc.tile_pool(name="recipp", bufs=2) as rp, \
             tc.tile_pool(name="ocopy", bufs=3) as oc, \
             tc.tile_pool(name="psS", bufs=2, space="PSUM") as psS, \
             tc.tile_pool(name="pscd", bufs=2, space="PSUM") as pscd:
            ctxT_sb = bw.tile([P, HL, S], BF16)     # [d, h, i]
            ow_sb = bw.tile([P, HL, E], BF16)
            for ho in range(HL):
                nc.sync.dma_start(ow_sb[:, ho, :], owT[ds(ho * P, P), :])
            for h in range(HL):
                for i in range(NS):
                    cd = pscd.tile([P, 2, ST], F32, tag="cd")
                    psc, psd = cd[:, 0, :], cd[:, 1, :]
                    for jp in range(JP):
                        megaS = psS.tile([P, 2, ST], F32, tag="megaS")
                        for s in range(2):
                            jb = 2 * jp + s
                            nc.tensor.matmul(
                                megaS[:, s, :],
                                qkT_sb[:, 2 * h + 1, ds(jb * P, P)],
                                qkT_sb[:, 2 * h, ds(i * ST, ST)],
                                start=True, stop=True)
                        att = pb.tile([P, 2, ST], BF16, tag="att")
                        nc.scalar.activation(
                            att[:], megaS[:], Act.Exp, scale=isc,
                            bias=negtwo[:])
                        for s in range(2):
                            jb = 2 * jp + s
                            nc.tensor.matmul(
                                psc[:], v_sb[:, jb, ds(h * D, D)],
                                att[:, s, :],
                                start=(jp == 0 and s == 0),
                                stop=(jp == JP - 1 and s == 1))
                            nc.tensor.matmul(
                                psd[:], ones_sb[:], att[:, s, :],
                                start=(jp == 0 and s == 0),
                                stop=(jp == JP - 1 and s == 1))
                    rec = rp.tile([P, ST], F32, tag="rec")
                    nc.vector.reciprocal(rec[:], psd[:])
                    nc.vector.tensor_tensor(
                        ctxT_sb[:, h, ds(i * ST, ST)], psc[:], rec[:],
                        Alu.mult)
                    # ---- Phase C for this seq chunk (all heads now done) --
                    if h == HL - 1:
                        for sb in range(4 * i, 4 * i + 4):
                            for et in range(ET):
                                psot = pscd.tile([P, 2, ST], F32,
                                                 tag="cd", name="psot")
                                pso = psot[:, 0, :]
                                for ho in range(HL):
                                    nc.tensor.matmul(
                                        pso[:],
                                        ctxT_sb[:, ho, ds(sb * P, P)],
                                        ow_sb[:, ho, ds(et * ST, ST)],
                                        start=(ho == 0), stop=(ho == HL - 1))
                                ot = oc.tile([P, ST], F32, tag="ot")
                                nc.vector.tensor_copy(ot[:], pso[:])
                                nc.sync.dma_start(
                                    out[ds(sb * P, P), ds(et * ST, ST)],
                                    ot[:])

    return nc


def _rope_tables():
    inv_freq = 1.0 / (10000.0 ** (np.arange(0, D, 2, dtype=np.float32) / D))
    t = np.arange(S, dtype=np.float32)
    freqs = np.einsum("s,f->sf", t, inv_freq)
    emb = np.concatenate([freqs, freqs], axis=-1)
    cosT = np.cos(emb).astype(np.float32).T.copy()
    sinT = np.sin(emb).astype(np.float32).T.copy()
    # fold the rotate-half sign in: out = qb*cos + halfswap(qb)*sinSW
    sinSW = np.concatenate([-sinT[:D // 2], sinT[D // 2:]], axis=0)
    return cosT, np.ascontiguousarray(sinSW)


def _core_inputs(Wqkv_w, Wqkv_b, out_w, g, cosT, sinT, xT_bf, xones, ones):
    qk_cols, qkb_rows = [], []
    for hl in range(HL):
        h = g * HL + hl
        qk_cols.append(Wqkv_w[h * D:(h + 1) * D, :].T)
        qk_cols.append(Wqkv_w[E + h * D:E + (h + 1) * D, :].T)
        qkb_rows.append(Wqkv_b[h * D:(h + 1) * D])
        qkb_rows.append(Wqkv_b[E + h * D:E + (h + 1) * D])
    wqkT = np.ascontiguousarray(np.concatenate(qk_cols, axis=1)).astype(BF)
    qkb = np.stack(qkb_rows).astype(np.float32)
    v0 = 2 * E + g * HL * D
    wvT = np.zeros((E + P, HL * D), dtype=np.float32)
    wvT[:E] = Wqkv_w[v0:v0 + HL * D, :].T
    wvT[E] = Wqkv_b[v0:v0 + HL * D]
    owT = np.ascontiguousarray(
        out_w[:, g * HL * D:(g + 1) * HL * D].T).astype(BF)
    return {"xT": xT_bf, "wqkT": wqkT, "qkb": qkb, "wvT": wvT.astype(BF),
            "xones": xones, "ones": ones,
            "cosT": cosT, "sinT": sinT, "owT": owT}


def kernel(x, Wqkv_w, Wqkv_b, out_w, out_b):
    global LAST_EXEC_NS
    _install_axon_ntff_shim()
    from concourse.bass_utils import run_bass_kernel_spmd

    x = np.asarray(x, dtype=np.float32)
    Wqkv_w = np.asarray(Wqkv_w, dtype=np.float32)
    Wqkv_b = np.asarray(Wqkv_b, dtype=np.float32)
    out_w = np.asarray(out_w, dtype=np.float32)
    out_b = np.asarray(out_b, dtype=np.float32)

    cosT, sinT = _rope_tables()
    xT_bf = [np.ascontiguousarray(x[b].T).astype(BF) for b in range(2)]
    xones = np.zeros((P, 512), dtype=BF)
    xones[0, :] = BF(1.0)
    ones = np.ones((P, P), dtype=BF)
    in_maps = []
    for core in range(8):
        b, g = core // 4, core % 4
        in_maps.append(_core_inputs(
            Wqkv_w, Wqkv_b, out_w, g, cosT, sinT, xT_bf[b], xones, ones))

    nc = bass.Bass()
    _build_mha(nc)
    _split_multi_waits(nc)

    trace = bool(os.environ.get("MHA_TRACE"))
    if trace:
        # dev-only profiling path; skip the S3 artifact upload
        import concourse.bass_utils as _bu
        _bu.upload_artifacts = lambda tmpdir: tmpdir
    res = run_bass_kernel_spmd(
        nc, in_maps, core_ids=list(range(8)), trace=trace)
    if trace:
        LAST_EXEC_NS = res.exec_time_ns

    out = np.empty((2, S, E), dtype=np.float32)
    for b in range(2):
        acc = res.results[b * 4 + 0]["out"].astype(np.float32).copy()
        for g in range(1, 4):
            acc += res.results[b * 4 + g]["out"]
        out[b] = acc + out_b[None, :]
    return out
